# revision 4
# baseline (speedup 1.0000x reference)
"""Trainium2 Bass kernel for nn_MultiHeadAttention (B=4, S=2048, D=1024, H=16).

Sharding: 8 cores; core c handles batch b=c//2, query-row half c%2 (1024 rows).
Each core computes K/V projections for its batch's full sequence (duplicated
across the pair of cores sharing a batch -> zero collectives), Q projection for
its own rows, all 16 heads of attention, and the output projection of its rows.

All compute is kept in "transposed" orientation (feature dim on partitions):
  qhT = Wq.T-chunks.T @ qT  -> [D, RQ]   (heads are 64-partition slices)
  khT = ...               -> [D, S]
  vh_aug = vTa.T @ WvTa    -> [S, H*65]  (per-head 64 cols + a ones column)
  scoresT[k,q] per head    -> matmul(lhsT=khT_h, rhs=qhT_h), K=64
  expT = exp(scoresT/8)    -> ACT, PSUM->SBUF
  attnT[d,q] (+denominator row from the ones column of vh_aug)
  concatT = attnT * (1/denominator) broadcast  (DRAM-bounce partition bcast)
  out = concatT-chunks.T @ WoT (+ bias via K=1 ones matmul)
so the softmax reduction runs along PSUM partitions via the matmul itself and
no on-device transposes are needed.  The host pre-transposes inputs/weights.
All matmuls run in float32r (full PE rate for moving dim >= 256).
"""
import math

import numpy as np

import concourse.bacc as bacc
import concourse.mybir as mybir
from concourse import tile
from concourse.bass_utils import run_bass_kernel_spmd

F32 = mybir.dt.float32
F32R = mybir.dt.float32r

B, S, D, H, HD = 4, 2048, 1024, 16, 64
NCORES = 8
RQ = S // 2          # query rows per core
QC = 512             # query-row chunk (psum bank width)
HA = H * 65          # vh_aug columns: per head 64 values + 1 ones column
P = 128


def build_nc(s=S, rq=RQ):
    kd = D // P              # feature chunks (contraction for projections)
    kt_n = s // P            # key-row tiles
    rt_n = s // P            # v rows tiles
    nqc = rq // QC
    L = 3                    # scores->attn software pipeline lag (in kt steps)

    nc = bacc.Bacc("TRN2", target_bir_lowering=False, num_devices=NCORES)

    qT = nc.declare_dram_parameter("qT", [D, rq], F32, isOutput=False)
    kT = nc.declare_dram_parameter("kT", [D, s], F32, isOutput=False)
    vTa = nc.declare_dram_parameter("vTa", [D + 1, s], F32, isOutput=False)
    wqT = nc.declare_dram_parameter("wqT", [D, D], F32, isOutput=False)
    wkT = nc.declare_dram_parameter("wkT", [D, D], F32, isOutput=False)
    wvTa = nc.declare_dram_parameter("wvTa", [D + 1, HA], F32, isOutput=False)
    woTa = nc.declare_dram_parameter("woTa", [D + 2, D], F32, isOutput=False)
    bq = nc.declare_dram_parameter("bq", [D, 1], F32, isOutput=False)
    bk = nc.declare_dram_parameter("bk", [D, 1], F32, isOutput=False)
    out = nc.declare_dram_parameter("out", [rq, D], F32, isOutput=True)

    def r32(ap):
        return ap.bitcast(F32R)

    with (
        nc.allow_low_precision(reason="fp32r matmul operand rounding"),
        tile.TileContext(nc) as tc,
    ):
        with (
            tc.tile_pool(name="vh", bufs=1) as p_vh,
            tc.tile_pool(name="qd", bufs=1, space="DRAM") as p_qd,
            tc.tile_pool(name="kh", bufs=1) as p_kh,
        ):
            vh = [p_vh.tile([P, HA], F32R, tag=f"vh{rt}", name=f"vh{rt}") for rt in range(rt_n)]
            qd = [p_qd.tile([P, rq], F32R, tag=f"qd{m}", name=f"qd{m}") for m in range(kd)]
            kh = [p_kh.tile([P, s], F32R, tag=f"kh{m}", name=f"kh{m}") for m in range(kd)]

            # ---------------- V projection (vh_aug = v_aug @ Wv_aug) --------
            with (
                tc.tile_pool(name="wv", bufs=1) as p_wv,
                tc.tile_pool(name="vt", bufs=4) as p_vt,
                tc.tile_pool(name="psv", bufs=2, space="PSUM") as p_psv,
            ):
                wv_t = [p_wv.tile([P, HA], F32R, tag=f"wv{k}", name=f"wv{k}") for k in range(kd)]
                for k in range(kd):
                    nc.sync.dma_start(wv_t[k][:], r32(wvTa.ap()[k * P:(k + 1) * P, :]))
                wv_last = p_wv.tile([1, HA], F32R, tag="wv8")
                nc.sync.dma_start(wv_last[:], r32(wvTa.ap()[D:D + 1, :]))

                nsplits = [(0, 512), (512, 1024), (1024, HA)]
                for rt in range(rt_n):
                    ps = p_psv.tile([P, HA], F32)
                    for k in range(kd + 1):
                        if k < kd:
                            lh = p_vt.tile([P, P], F32R, tag="vt")
                            nc.sync.dma_start(
                                lh[:], r32(vTa.ap()[k * P:(k + 1) * P, rt * P:(rt + 1) * P])
                            )
                            w = wv_t[k]
                            lhs = lh[:]
                        else:
                            lh = p_vt.tile([1, P], F32R, tag="vt8")
                            nc.sync.dma_start(
                                lh[:], r32(vTa.ap()[D:D + 1, rt * P:(rt + 1) * P])
                            )
                            w = wv_last
                            lhs = lh[:]
                        for (n0, n1) in nsplits:
                            nc.tensor.matmul(
                                ps[:, n0:n1], lhs, w[:, n0:n1],
                                start=(k == 0), stop=(k == kd),
                            )
                    nc.scalar.copy(vh[rt][:], ps[:])

            # ---------------- Q / K projections (transposed outputs) -------
            def proj_T(x_ap, w_ap, b_ap, writer, ncols, col_halves):
                """writer(m, rh_slice, psum_ap, bias_tile) stores one chunk."""
                groups = [(0, 1, 2), (3, 4, 5), (6, 7)]
                with (
                    tc.tile_pool(name="wx", bufs=4) as p_w,
                    tc.tile_pool(name="xt", bufs=3) as p_x,
                    tc.tile_pool(name="bb", bufs=1) as p_b,
                    tc.tile_pool(name="psp", bufs=4, space="PSUM") as p_ps,
                ):
                    b_t = [p_b.tile([P, 1], F32, tag=f"b{m}", name=f"b{m}") for m in range(kd)]
                    for m in range(kd):
                        nc.sync.dma_start(b_t[m][:], b_ap[m * P:(m + 1) * P, :])
                    for rh in range(col_halves):
                        cw = ncols // col_halves
                        for g in groups:
                            ps = {m: p_ps.tile([P, cw], F32, tag="psp", name=f"psp{m}") for m in g}
                            for k in range(kd):
                                xt = p_x.tile([P, cw], F32R, tag="xt")
                                nc.sync.dma_start(
                                    xt[:],
                                    r32(x_ap[k * P:(k + 1) * P, rh * cw:(rh + 1) * cw]),
                                )
                                for m in g:
                                    wt = p_w.tile([P, P], F32R, tag="wt")
                                    nc.sync.dma_start(
                                        wt[:],
                                        r32(w_ap[k * P:(k + 1) * P, m * P:(m + 1) * P]),
                                    )
                                    for n0 in range(0, cw, 512):
                                        nc.tensor.matmul(
                                            ps[m][:, n0:n0 + 512],
                                            wt[:],
                                            xt[:, n0:n0 + 512],
                                            start=(k == 0), stop=(k == kd - 1),
                                        )
                            for m in g:
                                writer(m, slice(rh * cw, (rh + 1) * cw), ps[m], b_t[m])

            with tc.tile_pool(name="qtmp", bufs=3) as p_qtmp:
                def q_writer(m, cols, ps, b_t):
                    qt = p_qtmp.tile([P, cols.stop - cols.start], F32R, tag="qtmp")
                    nc.scalar.activation(
                        qt[:], ps[:],
                        mybir.ActivationFunctionType.Identity, bias=b_t[:],
                    )
                    nc.sync.dma_start(qd[m][:, cols], qt[:])

                proj_T(qT.ap(), wqT.ap(), bq.ap(), q_writer, rq, max(1, rq // 1024))

            def k_writer(m, cols, ps, b_t):
                nc.scalar.activation(
                    kh[m][:, cols], ps[:],
                    mybir.ActivationFunctionType.Identity, bias=b_t[:],
                )

            proj_T(kT.ap(), wkT.ap(), bk.ap(), k_writer, s, max(1, s // 1024))

            # ---------------- attention + output projection -----------------
            with (
                tc.tile_pool(name="cc", bufs=1) as p_cc,
                tc.tile_pool(name="wo", bufs=2) as p_wo,
                tc.tile_pool(name="oc", bufs=1) as p_oc,
                tc.tile_pool(name="qs", bufs=2) as p_qs,
                tc.tile_pool(name="rc", bufs=3) as p_rc,
                tc.tile_pool(name="rb", bufs=3) as p_rb,
                tc.tile_pool(name="ob", bufs=3) as p_ob,
                tc.tile_pool(name="rd", bufs=4, space="DRAM") as p_rd,
            ):
                cc = [p_cc.tile([P, QC], F32R, tag=f"cc{m}", name=f"cc{m}") for m in range(kd)]
                ones_t = p_oc.tile([1, P], F32R, tag="ones")
                nc.sync.dma_start(ones_t[:], r32(woTa.ap()[D + 1:D + 2, 0:P]))
                bo_t = p_oc.tile([1, D], F32R, tag="bo")
                nc.sync.dma_start(bo_t[:], r32(woTa.ap()[D:D + 1, :]))

                for qc in range(nqc):
                    with (
                        tc.tile_pool(name="pss", bufs=L + 2, space="PSUM") as p_pss,
                        tc.tile_pool(name="psa", bufs=2, space="PSUM") as p_psa,
                        tc.tile_pool(name="ex", bufs=L + 2) as p_ex,
                    ):
                        for m in range(kd):
                          qs = p_qs.tile([P, QC], F32R, tag="qs")
                          nc.sync.dma_start(qs[:], qd[m][:, qc * QC:(qc + 1) * QC])
                          for hh in range(2):
                            h, off = 2 * m + hh, hh * HD
                            acc = p_psa.tile([HD + 1, QC], F32, tag="acc")
                            ex_t = {}
                            for step in range(kt_n + L):
                                if step < kt_n:
                                    kt = step
                                    pss = p_pss.tile([P, QC], F32, tag="pss")
                                    nc.tensor.matmul(
                                        pss[:],
                                        kh[m][off:off + HD, kt * P:(kt + 1) * P],
                                        qs[off:off + HD, :],
                                        start=True, stop=True,
                                    )
                                    ex = p_ex.tile([P, QC], F32R, tag="ex")
                                    nc.scalar.activation(
                                        ex[:], pss[:],
                                        mybir.ActivationFunctionType.Exp,
                                        scale=1.0 / math.sqrt(HD),
                                    )
                                    ex_t[kt] = ex
                                if step >= L:
                                    j = step - L
                                    nc.tensor.matmul(
                                        acc[:],
                                        vh[j][:, h * 65:h * 65 + 65],
                                        ex_t.pop(j)[:],
                                        start=(j == 0), stop=(j == kt_n - 1),
                                    )
                            # normalization: r = 1/acc[64]; bcast via DRAM
                            rc = p_rc.tile([1, QC], F32, tag="rc")
                            nc.vector.reciprocal(rc[:], acc[HD:HD + 1, :])
                            rd = p_rd.tile([1, QC], F32)
                            nc.sync.dma_start(rd[:], rc[:])
                            rb = p_rb.tile([HD, QC], F32, tag="rb")
                            nc.sync.dma_start(rb[:], rd[0:1, :].to_broadcast((HD, QC)))
                            nc.vector.tensor_tensor(
                                cc[m][off:off + HD, :], acc[0:HD, :], rb[:],
                                mybir.AluOpType.mult,
                            )

                    # ---- output projection for this query chunk ----
                    with tc.tile_pool(name="pso", bufs=1, space="PSUM") as p_pso:
                        rt4 = QC // P
                        pso = {
                            (rt, n2): p_pso.tile([P, 512], F32, tag=f"o{rt}_{n2}", name=f"o{rt}_{n2}")
                            for rt in range(rt4) for n2 in range(2)
                        }
                        for d in range(kd):
                            wo_t = p_wo.tile([P, D], F32R, tag="wo")
                            nc.sync.dma_start(
                                wo_t[:], r32(woTa.ap()[d * P:(d + 1) * P, :])
                            )
                            for rt in range(rt4):
                                for n2 in range(2):
                                    nc.tensor.matmul(
                                        pso[(rt, n2)][:],
                                        cc[d][:, rt * P:(rt + 1) * P],
                                        wo_t[:, n2 * 512:(n2 + 1) * 512],
                                        start=(d == 0), stop=False,
                                    )
                        for rt in range(rt4):
                            for n2 in range(2):
                                nc.tensor.matmul(
                                    pso[(rt, n2)][:],
                                    ones_t[0:1, :],
                                    bo_t[0:1, n2 * 512:(n2 + 1) * 512],
                                    start=False, stop=True,
                                )
                                ob = p_ob.tile([P, 512], F32, tag="ob")
                                nc.scalar.copy(ob[:], pso[(rt, n2)][:])
                                nc.sync.dma_start(
                                    out.ap()[
                                        qc * QC + rt * P:qc * QC + (rt + 1) * P,
                                        n2 * 512:(n2 + 1) * 512,
                                    ],
                                    ob[:],
                                )

    nc.compile()
    return nc


def prep_core_inputs(q, k, v, Wq, bq, Wk, bk, Wv, bv, Wo, bo, s=S, rq=RQ):
    """Build the per-core input maps (host-side shard + transpose + augment)."""
    f = np.float32
    wqT = np.ascontiguousarray(np.asarray(Wq, f).T)
    wkT = np.ascontiguousarray(np.asarray(Wk, f).T)
    woTa = np.concatenate(
        [np.asarray(Wo, f).T, np.asarray(bo, f).reshape(1, D), np.ones((1, D), f)],
        axis=0,
    )
    woTa = np.ascontiguousarray(woTa)
    wvT = np.asarray(Wv, f).T
    wvTa = np.zeros((D + 1, HA), f)
    for h in range(H):
        wvTa[0:D, h * 65:h * 65 + HD] = wvT[:, h * HD:(h + 1) * HD]
        wvTa[D, h * 65:h * 65 + HD] = np.asarray(bv, f)[h * HD:(h + 1) * HD]
        wvTa[D, h * 65 + HD] = 1.0
    bqc = np.ascontiguousarray(np.asarray(bq, f).reshape(D, 1))
    bkc = np.ascontiguousarray(np.asarray(bk, f).reshape(D, 1))

    n_cores = (np.asarray(q).shape[0] * np.asarray(q).shape[1]) // rq
    in_maps = []
    ones_row = np.ones((1, s), f)
    for c in range(n_cores):
        b, half = divmod(c, max(1, n_cores // np.asarray(q).shape[0]))
        qT_c = np.ascontiguousarray(np.asarray(q, f)[b, half * rq:(half + 1) * rq, :].T)
        kT_c = np.ascontiguousarray(np.asarray(k, f)[b].T)
        vTa_c = np.ascontiguousarray(
            np.concatenate([np.asarray(v, f)[b].T, ones_row], axis=0)
        )
        in_maps.append({
            "qT": qT_c, "kT": kT_c, "vTa": vTa_c,
            "wqT": wqT, "wkT": wkT, "wvTa": wvTa, "woTa": woTa,
            "bq": bqc, "bk": bkc,
        })
    return in_maps


_NC_CACHE = {}


def run(q, k, v, Wq, bq, Wk, bk, Wv, bv, Wo, bo, trace=False):
    key = ("full", S, RQ)
    if key not in _NC_CACHE:
        _NC_CACHE[key] = build_nc(S, RQ)
    nc = _NC_CACHE[key]
    in_maps = prep_core_inputs(q, k, v, Wq, bq, Wk, bk, Wv, bv, Wo, bo)
    res = run_bass_kernel_spmd(nc, in_maps, list(range(NCORES)), trace=trace)
    Bq, Sq, Dq = np.asarray(q).shape
    full = np.empty((Bq, Sq, Dq), np.float32)
    per_b = NCORES // Bq
    for c in range(NCORES):
        b, half = divmod(c, per_b)
        full[b, half * RQ:(half + 1) * RQ, :] = res.results[c]["out"]
    return full, res


def kernel(q, k, v, Wq, bq, Wk, bk, Wv, bv, Wo, bo):
    full, _ = run(q, k, v, Wq, bq, Wk, bk, Wv, bv, Wo, bo, trace=False)
    return full
